# revision 1
# baseline (speedup 1.0000x reference)
"""HSMNet cost-volume + disparity softmax-regression on 8 Trainium2 NeuronCores.

Reference computation (per batch b):
  cost[c,d,h,w] = |ref[c,h,w] - tgt[c,h,w-d]| for w>=d else 0
  cost_agg[d,h,w] = sum_c cost
  pred[h,w] = sum_d d * softmax_d(cost_agg)

Sharding: 8 cores = 4 batches x 2 h-halves (40 rows of 80 each). Each core
processes its [32, 40, 160] slice fully fused on-chip:
  - pixels flattened to 6400; disparity handled as 6 blocks of 4 d's packed
    with the 32 channels into 128 SBUF partitions (partition = c + 32*j,
    d = 4*blk + j). tgt is replicated into 4 partition groups with baked-in
    shift j (front zero-padded), so one DVE tensor_tensor subtract with a
    uniform column offset produces diffs for 4 disparities at once.
  - abs via uint16 bitcast & 0x7fff (DVE 4x) / ACT Abs (configurable split)
  - channel reduction via TensorE matmul with a 0/1 lhsT -> PSUM [24, *]
  - softmax: ACT Exp evacuates PSUM -> E[96,1600] bf16 (quarters of the
    pixel range stacked on partitions), DVE multiplies by the validity mask
    (w >= d), TensorE contracts with [ones; d] weights -> den/num [8, 1600]
  - host divides num/den (the invalid entries' exp(0)=1 terms are dropped;
    they are < 1e-5 of den for randn-scale inputs)
"""
import os
import sys
import threading

for _p in ("/opt/trn_rl_repo",):
    if os.path.isdir(_p) and _p not in sys.path:
        sys.path.insert(0, _p)

import numpy as np
import ml_dtypes

import concourse.bacc as bacc
import concourse.mybir as mybir
from concourse.tile import TileContext
from concourse.bass_utils import run_bass_kernel_spmd

dt = mybir.dt

# problem shape (hardcoded per spec)
B, C, H, W = 4, 32, 80, 160
D = 24
HP = H // 2            # rows per core
PIX = HP * W           # 6400 pixels per core
HALF = PIX // 2        # 3200
NB = D // 4            # 6 disparity blocks of 4
NQ = 4                 # pixel quarters on E partitions
QW = PIX // NQ         # 1600
CH = 400               # matmul chunk (512-aligned in PSUM)
PAD = 24               # zero pad columns in front of tgt_rep
N_CORES = 8

# abs engine per (half, block) index 0..11: "dve" = uint16 bitand (4x mode),
# "act" = scalar engine Abs, "stt" = DVE max(-x,x) (1-port, gpsimd-safe)
ABS_ENGINES = os.environ.get("HSM_ABS", "dve,act,dve,act,dve,act,dve,act,dve,act,dve,act").split(",")
CAST_ENGINE = os.environ.get("HSM_CAST", "act")  # "act" | "gps" | "dve"
DIFF_BUFS = int(os.environ.get("HSM_DIFF_BUFS", "16"))
STAGE = int(os.environ.get("HSM_STAGE", "3"))  # 1=diff only, 2=+cost/exp/mask, 3=full


def _build_program():
    nc = bacc.Bacc("TRN2", target_bir_lowering=False)
    ref_h = nc.dram_tensor("ref", [C, PIX], dt.float32, kind="ExternalInput")
    tgt_h = nc.dram_tensor("tgt", [C, PIX], dt.float32, kind="ExternalInput")
    mask_h = nc.dram_tensor("mask", [128, QW], dt.bfloat16, kind="ExternalInput")
    lred_h = nc.dram_tensor("lred", [128, NB * D], dt.float16, kind="ExternalInput")
    lnd_h = nc.dram_tensor("lnd", [128, 8], dt.bfloat16, kind="ExternalInput")
    out_h = nc.dram_tensor("out", [8, NQ * CH], dt.float32, kind="ExternalOutput")

    with TileContext(nc) as tc:
        with tc.tile_pool(name="const", bufs=1) as cpool, \
             tc.tile_pool(name="stage", bufs=1) as spool, \
             tc.tile_pool(name="rep", bufs=1) as rpool, \
             tc.tile_pool(name="diffp", bufs=DIFF_BUFS) as dpool, \
             tc.tile_pool(name="ep", bufs=1) as epool:
            mask_sb = cpool.tile([128, QW], dt.bfloat16)
            lred_sb = cpool.tile([128, NB * D], dt.float16)
            lnd_sb = cpool.tile([128, 8], dt.bfloat16)
            nc.sync.dma_start(lred_sb[:], lred_h[:])

            stage32 = spool.tile([64, PIX], dt.float32)
            f16s = spool.tile([64, PIX], dt.float16)
            ref_rep = rpool.tile([128, PIX], dt.float16)
            tgt_rep = rpool.tile([128, PAD + PIX], dt.float16)
            E = epool.tile([128, QW], dt.bfloat16)

            # zero the leading pad (covers cols [0, 24+j) for every group j)
            nc.vector.memset(tgt_rep[:, 0:PAD + 4], 0.0)
            # zero E pad rows (24-31 of each 32-row quarter group) so the
            # num/den matmul never touches uninitialized SBUF
            nc.vector.memset(E[:], 0.0)

            with tc.tile_pool(name="cost", bufs=2, space="PSUM") as qpool:
                # all HBM loads up front on the sync queue (no head-of-line
                # blocking behind sem-waiting replicate DMAs)
                for q in range(NQ):
                    c0, c1 = QW * q, QW * (q + 1)
                    nc.sync.dma_start(stage32[0:32, c0:c1], ref_h[:, c0:c1])
                    nc.sync.dma_start(stage32[32:64, c0:c1], tgt_h[:, c0:c1])
                nc.sync.dma_start(mask_sb[:], mask_h[:])
                nc.sync.dma_start(lnd_sb[:], lnd_h[:])
                for q in range(NQ):
                    c0, c1 = QW * q, QW * (q + 1)
                    if CAST_ENGINE == "act":
                        nc.scalar.activation(f16s[0:32, c0:c1], stage32[0:32, c0:c1],
                                             mybir.ActivationFunctionType.Copy)
                        nc.scalar.activation(f16s[32:64, c0:c1], stage32[32:64, c0:c1],
                                             mybir.ActivationFunctionType.Copy)
                    elif CAST_ENGINE == "gps":
                        nc.gpsimd.tensor_copy(f16s[0:32, c0:c1], stage32[0:32, c0:c1])
                        nc.gpsimd.tensor_copy(f16s[32:64, c0:c1], stage32[32:64, c0:c1])
                    else:
                        nc.vector.tensor_copy(f16s[:, c0:c1], stage32[:, c0:c1])
                    for j in range(4):
                        nc.sync.dma_start(ref_rep[32 * j:32 * j + 32, c0:c1],
                                            f16s[0:32, c0:c1])
                    for j in range(4):
                        # tgt_rep[c+32j, s] = tgt[c, s - PAD - j]
                        d_lo = PAD + j + c0
                        d_hi = PAD + PIX if q == NQ - 1 else PAD + j + c1
                        s_hi = (PIX - j) if q == NQ - 1 else c1
                        nc.sync.dma_start(tgt_rep[32 * j:32 * j + 32, d_lo:d_hi],
                                            f16s[32:64, c0:s_hi])

                    diffs = []
                    for b in range(NB):
                        diff = dpool.tile([128, QW], dt.float16, tag="diff",
                                          name=f"diff_{q}_{b}")
                        # diff[c+32j, p] = ref[c, p] - tgt[c, p - 4b - j]
                        nc.vector.tensor_tensor(
                            diff[:], ref_rep[:, c0:c1],
                            tgt_rep[:, PAD - 4 * b + c0:PAD - 4 * b + c1],
                            mybir.AluOpType.subtract)
                        eng = ABS_ENGINES[(q * NB + b) % len(ABS_ENGINES)]
                        if eng == "dve":
                            du = diff[:].bitcast(dt.uint16)
                            nc.vector.tensor_scalar(du, du, 0x7FFF, None,
                                                    mybir.AluOpType.bitwise_and)
                        elif eng == "act":
                            nc.scalar.activation(diff[:], diff[:],
                                                 mybir.ActivationFunctionType.Abs)
                        else:  # stt: |x| = max(-x, x), 1-port DVE
                            nc.vector.scalar_tensor_tensor(
                                diff[:], diff[:], -1.0, diff[:],
                                op0=mybir.AluOpType.mult, op1=mybir.AluOpType.max)
                        diffs.append(diff)

                    if STAGE < 2:
                        continue
                    cost = qpool.tile([D, 2048], dt.float32, tag="cost",
                                      name=f"cost_{q}")
                    for b in range(NB):
                        for cc in range(4):
                            nc.tensor.matmul(
                                cost[:, 512 * cc:512 * cc + CH],
                                lred_sb[:, D * b:D * (b + 1)],
                                diffs[b][:, CH * cc:CH * cc + CH],
                                start=(b == 0), stop=(b == NB - 1))
                    # exp evacuate PSUM -> E bf16 (strided 512 -> packed 400)
                    src = cost[:].rearrange("p (k x) -> p k x", k=4)[:, :, 0:CH]
                    dst = E[32 * q:32 * q + D, :].rearrange("p (k x) -> p k x", x=CH)
                    nc.scalar.activation(dst, src, mybir.ActivationFunctionType.Exp)
                    # zero invalid entries (w < d) for this quarter's rows
                    r0, r1 = 32 * q, 32 * (q + 1)
                    nc.vector.tensor_tensor(E[r0:r1, :], E[r0:r1, :],
                                            mask_sb[r0:r1, :], mybir.AluOpType.mult)

            if STAGE >= 3:
                with tc.tile_pool(name="nd", bufs=1, space="PSUM") as npool:
                    nd = npool.tile([8, 2048], dt.float32)
                    for cc in range(4):
                        nc.tensor.matmul(nd[:, 512 * cc:512 * cc + CH],
                                         lnd_sb[:], E[:, CH * cc:CH * (cc + 1)],
                                         start=True, stop=True)
                    ndsrc = nd[:].rearrange("p (k x) -> p k x", k=4)[:, :, 0:CH]
                    out_sb = epool.tile([8, NQ * CH], dt.float32)
                    nc.scalar.activation(
                        out_sb[:].rearrange("p (k x) -> p k x", x=CH), ndsrc,
                        mybir.ActivationFunctionType.Copy)
                    nc.sync.dma_start(out_h[:], out_sb[:])
            else:
                out_sb = epool.tile([8, NQ * CH], dt.float32)
                src = E[0:8, :] if STAGE == 2 else None
                if STAGE == 1:
                    nc.scalar.activation(out_sb[:], tgt_rep[0:8, 0:NQ * CH],
                                         mybir.ActivationFunctionType.Copy)
                else:
                    nc.scalar.activation(out_sb[:], src,
                                         mybir.ActivationFunctionType.Copy)
                nc.sync.dma_start(out_h[:], out_sb[:])

    nc.compile()
    return nc


def _host_constants():
    w = np.arange(W, dtype=np.int64)
    dvals = np.arange(D, dtype=np.int64)
    # mask[d + 32q, n] = 1 if (n mod 160) >= d; rows 24-31 of each group = 0
    m = (np.tile(w, QW // W)[None, :] >= dvals[:, None]).astype(np.float32)  # [24, 1600]
    mask = np.zeros((128, QW), np.float32)
    for q in range(4):
        mask[32 * q:32 * q + D, :] = m
    mask = mask.astype(ml_dtypes.bfloat16)

    lred = np.zeros((128, NB * D), np.float16)
    for b in range(NB):
        for j in range(4):
            for c in range(C):
                lred[c + 32 * j, D * b + 4 * b + j] = 1.0

    lnd = np.zeros((128, 8), np.float32)
    for q in range(4):
        for d in range(D):
            lnd[d + 32 * q, q] = 1.0      # den
            lnd[d + 32 * q, 4 + q] = d    # num
    lnd = lnd.astype(ml_dtypes.bfloat16)
    return mask, lred, lnd


_lock = threading.Lock()
_cache = {}


def _get_program():
    with _lock:
        if "nc" not in _cache:
            _cache["nc"] = _build_program()
            _cache["consts"] = _host_constants()
        return _cache["nc"], _cache["consts"]


def _run(refimg_fea, targetimg_fea, trace=False):
    nc, (mask, lred, lnd) = _get_program()
    ref = np.ascontiguousarray(refimg_fea, dtype=np.float32)
    tgt = np.ascontiguousarray(targetimg_fea, dtype=np.float32)
    in_maps = []
    for core in range(N_CORES):
        b, hh = core // 2, core % 2
        in_maps.append({
            "ref": ref[b, :, HP * hh:HP * (hh + 1), :].reshape(C, PIX).copy(),
            "tgt": tgt[b, :, HP * hh:HP * (hh + 1), :].reshape(C, PIX).copy(),
            "mask": mask, "lred": lred, "lnd": lnd,
        })
    res = run_bass_kernel_spmd(nc, in_maps, core_ids=list(range(N_CORES)),
                               trace=trace)
    out = np.empty((B, H, W), np.float32)
    for core in range(N_CORES):
        b, hh = core // 2, core % 2
        nd = res.results[core]["out"]          # [8, 1600]: den q rows 0-3, num rows 4-7
        pred = nd[4:8] / nd[0:4]               # [4, 1600]
        out[b, HP * hh:HP * (hh + 1), :] = pred.reshape(HP, W)
    return out, res


def kernel(refimg_fea, targetimg_fea, maxdisp):
    assert int(maxdisp) == D, f"kernel hardcodes maxdisp={D}, got {maxdisp}"
    out, _ = _run(refimg_fea, targetimg_fea)
    return out



# revision 11
# speedup vs baseline: 1.4325x; 1.4325x over previous
"""HSMNet cost-volume + disparity softmax-regression on 8 Trainium2 NeuronCores.

Reference computation (per batch b):
  cost[c,d,h,w] = |ref[c,h,w] - tgt[c,h,w-d]| for w>=d else 0
  cost_agg[d,h,w] = sum_c cost
  pred[h,w] = sum_d d * softmax_d(cost_agg)

Sharding: 8 cores = 4 batches x 2 h-halves (40 rows of 80 each). Each core
processes its [32, 40, 160] slice fully fused on-chip.

Host prep (layout only, no arithmetic): inputs are cast to fp16 and
replicated into 4 partition groups (partition = c + 32*j) with the shift j
baked into tgt via a 24-col front zero pad. On-chip, per eighth of the
pixel range (800 pixels):
  - one DVE tensor_tensor subtract with a 3D access pattern (disparity
    block dim stride +4 on tgt, stride 0 broadcast on ref) produces diffs
    for all 24 disparities: diff[c+32j, k, p] = ref[c,p] - tgt[c, p-4b-j],
    b = 5-k.
  - abs in place, split across DVE (uint16 bitand), ACT (Abs), GPSIMD
    (uint16 bitand) per env-tunable column split.
  - TensorE reduces channels with 0/1 weights into PSUM [24, 2x512], plus
    one extra accumulation matmul that adds -10000 where w < d (validity
    mask folded into the PE pass: [w<d] = sum_k [k<d]*[w==k]).
  - ACT Exp evacuates PSUM -> E[96, 1600] bf16 (rows 24q+d).
  - TensorE contracts E with [ones; d] weights -> den/num [8, 1600].
  - host divides num/den (invalid entries' terms vanish: exp(-1e4) = 0).
"""
import os
import sys
import threading

for _p in ("/opt/trn_rl_repo",):
    if os.path.isdir(_p) and _p not in sys.path:
        sys.path.insert(0, _p)

import numpy as np
import ml_dtypes

import concourse.bacc as bacc
import concourse.mybir as mybir
from concourse.tile import TileContext
from concourse.bass_utils import run_bass_kernel_spmd

dt = mybir.dt

# problem shape (hardcoded per spec)
B, C, H, W = 4, 32, 80, 160
D = 24
HP = H // 2            # rows per core
PIX = HP * W           # 6400 pixels per core
NB = D // 4            # 6 disparity blocks of 4
PAD = 24               # zero pad columns in front of tgtr
NE = 8                 # processing units (eighths of the pixel range)
EW = PIX // NE         # 800 pixels per eighth
QW = PIX // 4          # 1600 pixels per quarter (E column range)
N_CORES = 8

# abs column split within each [128, 4800] diff tile: [0:A) DVE bitand,
# [A:B) ACT Abs, [B:4800) GPSIMD bitand. Multiples of 16.
ABS_DVE = int(os.environ.get("HSM_ABS_DVE", "2400"))
ABS_ACT = int(os.environ.get("HSM_ABS_ACT", "2400"))
DIFF_BUFS = int(os.environ.get("HSM_DIFF_BUFS", "3"))
OUT_VIA_ACT = int(os.environ.get("HSM_OUT_ACT", "1"))


def _build_program():
    nc = bacc.Bacc("TRN2", target_bir_lowering=False)
    refr_h = nc.dram_tensor("refr", [128, PIX], dt.float16, kind="ExternalInput")
    tgtr_h = nc.dram_tensor("tgtr", [128, PAD + PIX], dt.float16,
                            kind="ExternalInput")
    lred_h = nc.dram_tensor("lred", [128, NB * D], dt.float16,
                            kind="ExternalInput")
    lmask_h = nc.dram_tensor("lmask", [D, D], dt.float16, kind="ExternalInput")
    maskc_h = nc.dram_tensor("maskc", [D, EW], dt.float16, kind="ExternalInput")
    lnd_h = nc.dram_tensor("lnd", [128, 8], dt.bfloat16, kind="ExternalInput")
    out_h = nc.dram_tensor("out", [8, 4 * 400], dt.float32, kind="ExternalOutput")

    with TileContext(nc) as tc:
        with tc.tile_pool(name="const", bufs=1) as cpool, \
             tc.tile_pool(name="inp", bufs=1) as ipool, \
             tc.tile_pool(name="diffp", bufs=DIFF_BUFS) as dpool, \
             tc.tile_pool(name="ep", bufs=1) as epool:
            lred_sb = cpool.tile([128, NB * D], dt.float16)
            lmask_sb = cpool.tile([D, D], dt.float16)
            maskc_sb = cpool.tile([D, EW], dt.float16)
            lnd_sb = cpool.tile([128, 8], dt.bfloat16)
            nc.sync.dma_start(lred_sb[:], lred_h[:])
            nc.sync.dma_start(lmask_sb[:], lmask_h[:])
            nc.sync.dma_start(maskc_sb[:], maskc_h[:])
            nc.sync.dma_start(lnd_sb[:], lnd_h[:])

            refr = ipool.tile([128, PIX], dt.float16)
            tgtr = ipool.tile([128, PAD + PIX], dt.float16)
            E = epool.tile([128, QW], dt.bfloat16)
            # rows 32q+24..32q+31 are never written by the exp evac; zero
            # them once so the num/den matmul sees 0 (their lnd weights are
            # 0, but garbage could be inf/nan)
            nc.vector.memset(E[:], 0.0)

            def emit_load(e):
                c0 = EW * e
                nc.sync.dma_start(refr[:, c0:c0 + EW], refr_h[:, c0:c0 + EW])
                t0 = 0 if e == 0 else PAD + c0
                t1 = PAD + c0 + EW
                nc.sync.dma_start(tgtr[:, t0:t1], tgtr_h[:, t0:t1])

            diffs = {}

            def emit_tt(e):
                c0 = EW * e
                diff = dpool.tile([128, NB * EW], dt.float16, tag="diff",
                                  name=f"diff_{e}")
                out = diff[:].rearrange("p (k x) -> p k x", x=EW)
                in0 = refr[:, c0:c0 + EW].unsqueeze(1).broadcast_to(
                    [128, NB, EW])
                # tgt windows: block k reads cols [c0+4+4k, c0+4+4k+EW)
                # -> diff slot k holds disparity block b = 5-k
                in1 = tgtr[:, c0 + 4:c0 + 4 + EW].unsqueeze(1).broadcast_to(
                    [128, NB, EW]).copy()
                in1.ap = in1.ap[:1] + (((4, NB)),) + in1.ap[2:]
                nc.vector.tensor_tensor(out, in0, in1,
                                        mybir.AluOpType.subtract)
                diffs[e] = diff

            def emit_abs(e):
                diff = diffs[e]
                if ABS_DVE > 0:
                    du = diff[:, 0:ABS_DVE].bitcast(dt.uint16)
                    nc.vector.tensor_scalar(du, du, 0x7FFF, None,
                                            mybir.AluOpType.bitwise_and)
                if ABS_ACT > 0:
                    s0, s1 = ABS_DVE, ABS_DVE + ABS_ACT
                    nc.scalar.activation(diff[:, s0:s1], diff[:, s0:s1],
                                         mybir.ActivationFunctionType.Abs)
                if ABS_DVE + ABS_ACT < NB * EW:
                    s0 = ABS_DVE + ABS_ACT
                    g = diff[:, s0:NB * EW]
                    nc.gpsimd.scalar_tensor_tensor(
                        g, g, -1.0, g,
                        op0=mybir.AluOpType.mult, op1=mybir.AluOpType.max)

            costs = {}

            def emit_pe(e, qpool):
                diff = diffs[e]
                cost = qpool.tile([D, 1024], dt.float32, tag="cost",
                                  name=f"cost_{e}")
                for k in range(NB):
                    b = NB - 1 - k
                    for cc in range(2):
                        nc.tensor.matmul(
                            cost[:, 512 * cc:512 * cc + 400],
                            lred_sb[:, D * b:D * (b + 1)],
                            diff[:, EW * k + 400 * cc:EW * k + 400 * cc + 400],
                            start=(k == 0), stop=False)
                for cc in range(2):
                    nc.tensor.matmul(
                        cost[:, 512 * cc:512 * cc + 400],
                        lmask_sb[:],
                        maskc_sb[:, 400 * cc:400 * cc + 400],
                        start=False, stop=(cc == 1))
                costs[e] = cost

            def emit_exp(e):
                q, hh = e // 2, e % 2
                cost = costs[e]
                src = cost[:].rearrange("p (k x) -> p k x", x=512)[:, :, 0:400]
                dst = E[32 * q:32 * q + D,
                        EW * hh:EW * (hh + 1)].rearrange(
                            "p (k x) -> p k x", x=400)
                nc.scalar.activation(dst, src, mybir.ActivationFunctionType.Exp)
                del costs[e], diffs[e]

            with tc.tile_pool(name="cost", bufs=4, space="PSUM") as qpool:
                for e in range(NE):
                    emit_load(e)
                    emit_tt(e)
                    emit_abs(e)
                    if e >= 1:
                        emit_pe(e - 1, qpool)
                        emit_exp(e - 1)
                emit_pe(NE - 1, qpool)
                emit_exp(NE - 1)

            with tc.tile_pool(name="nd", bufs=1, space="PSUM") as npool:
                nd = npool.tile([8, 2048], dt.float32)
                for cc in range(4):
                    nc.tensor.matmul(nd[:, 512 * cc:512 * cc + 400],
                                     lnd_sb[:], E[:, 400 * cc:400 * (cc + 1)],
                                     start=True, stop=True)
                ndsrc = nd[:].rearrange("p (k x) -> p k x", x=512)[:, :, 0:400]
                if OUT_VIA_ACT:
                    out_sb = epool.tile([8, 4 * 400], dt.float32)
                    nc.scalar.activation(
                        out_sb[:].rearrange("p (k x) -> p k x", x=400), ndsrc,
                        mybir.ActivationFunctionType.Copy)
                    nc.sync.dma_start(out_h[:], out_sb[:])
                else:
                    nc.sync.dma_start(
                        out_h[:].rearrange("p (k x) -> p k x", x=400), ndsrc)

    nc.compile()
    return nc


def _host_constants():
    # lred: block b sums channels of partition group j into cost row 4b+j
    lred = np.zeros((128, NB * D), np.float16)
    for b in range(NB):
        for j in range(4):
            for c in range(C):
                lred[c + 32 * j, D * b + 4 * b + j] = 1.0

    # bias[d, p] = sum_k lmask[k, d] * maskc[k, p] = -1e4 * [(p mod W) < d]
    lmask = np.zeros((D, D), np.float16)
    for k in range(D):
        for d in range(D):
            if k < d:
                lmask[k, d] = 1.0
    maskc = np.zeros((D, EW), np.float16)
    for k in range(D):
        maskc[k, np.arange(EW)[np.arange(EW) % W == k]] = -10000.0

    lnd = np.zeros((128, 8), np.float32)
    for q in range(4):
        for d in range(D):
            lnd[d + 32 * q, q] = 1.0      # den
            lnd[d + 32 * q, 4 + q] = d    # num
    lnd = lnd.astype(ml_dtypes.bfloat16)
    return lred, lmask, maskc, lnd


_lock = threading.Lock()
_cache = {}


def _get_program():
    with _lock:
        if "nc" not in _cache:
            _cache["nc"] = _build_program()
            _cache["consts"] = _host_constants()
        return _cache["nc"], _cache["consts"]


def _prep_core(ref_s, tgt_s):
    """ref_s, tgt_s: [32, 6400] fp16 -> replicated tiles."""
    refr = np.ascontiguousarray(
        np.broadcast_to(ref_s[None], (4, C, PIX)).reshape(128, PIX))
    tgtr = np.zeros((128, PAD + PIX), np.float16)
    for j in range(4):
        tgtr[32 * j:32 * j + 32, PAD + j:] = tgt_s[:, :PIX - j]
    return refr, tgtr


def _run(refimg_fea, targetimg_fea, trace=False):
    nc, (lred, lmask, maskc, lnd) = _get_program()
    ref = np.asarray(refimg_fea, dtype=np.float32).astype(np.float16)
    tgt = np.asarray(targetimg_fea, dtype=np.float32).astype(np.float16)
    in_maps = []
    for core in range(N_CORES):
        b, hh = core // 2, core % 2
        ref_s = ref[b, :, HP * hh:HP * (hh + 1), :].reshape(C, PIX)
        tgt_s = tgt[b, :, HP * hh:HP * (hh + 1), :].reshape(C, PIX)
        refr, tgtr = _prep_core(ref_s, tgt_s)
        in_maps.append({
            "refr": refr, "tgtr": tgtr,
            "lred": lred, "lmask": lmask, "maskc": maskc, "lnd": lnd,
        })
    res = run_bass_kernel_spmd(nc, in_maps, core_ids=list(range(N_CORES)),
                               trace=trace)
    out = np.empty((B, H, W), np.float32)
    for core in range(N_CORES):
        b, hh = core // 2, core % 2
        nd = res.results[core]["out"]          # [8, 1600]: den rows 0-3, num 4-7
        pred = nd[4:8] / nd[0:4]               # [4, 1600]
        out[b, HP * hh:HP * (hh + 1), :] = pred.reshape(HP, W)
    return out, res


def kernel(refimg_fea, targetimg_fea, maxdisp):
    assert int(maxdisp) == D, f"kernel hardcodes maxdisp={D}, got {maxdisp}"
    out, _ = _run(refimg_fea, targetimg_fea)
    return out
